# revision 11
# baseline (speedup 1.0000x reference)
"""Trainium2 Bass kernel for DependencyGNN (2-layer GCN + global mean pool).

Distribution (8 cores, SPMD):
  - Nodes sharded contiguously (25000/core, padded to 25088 = 196*128).
  - Stage 1: h~ = dinv * (x_shard @ W1) in bf16, kept in SBUF and written to
    DRAM in 4 chunks; each chunk is AllGathered (bf16) while stage 1 continues.
  - Layer 1 (dst-sharded edges): batched indirect gathers of h~[src] from the
    AG table; one-hot (is_equal) selection matmuls accumulate per 128-dst
    window into PSUM; self-loop added via identity matmul; epilogue
    relu(dinv*acc + b1) -> u table (bf16, local DRAM + kernel output).
  - Layer 2 + mean-pool fused: out-edge (src-local) gathers of u, coefficient
    dinv[dst]*dinv[src], one-hot matmul onto per-graph windows -> per-core
    partial pooled sums (f32 output). No second collective.
  - Host: self-loop pooling terms from u, sum partials, /counts, @W2 + b2.
"""
import numpy as np

N_NODES = 200000
N_EDGES = 400000
IN_CH, HID_CH, OUT_CH = 768, 256, 256
NUM_GRAPHS = 8000
N_CORES = 8
P = 128
NPC = N_NODES // N_CORES            # 25000
NT = (NPC + P - 1) // P             # 196 m-tiles per core
NPC_PAD = NT * P                    # 25088
K_TILES = IN_CH // P                # 6
NCH = 4                             # AllGather chunks
CHM = NT // NCH                     # 49 m-tiles per chunk
CH_ROWS = CHM * P                   # 6272 rows per chunk
NGW = (NUM_GRAPHS + P - 1) // P     # 63 graph windows (8064 padded)
NG_PAD = NGW * P
GB = 16                             # gather batch (tiles per indirect DMA)
MB = 8                              # stage-1 m-tile DMA batch / write batch
WB = 4                              # epilogue window batch (PSUM group)


def _bf16():
    import concourse.mybir as mybir
    return mybir.dt.np(mybir.dt.bfloat16)


def _prep(x, W1, b1, W2, b2, edge_index, batch):
    bf16 = _bf16()
    x = np.asarray(x, dtype=np.float32)
    W1 = np.asarray(W1, dtype=np.float32)
    b1 = np.asarray(b1, dtype=np.float32)
    W2 = np.asarray(W2, dtype=np.float32)
    b2 = np.asarray(b2, dtype=np.float32)
    src = np.asarray(edge_index[0], dtype=np.int64)
    dst = np.asarray(edge_index[1], dtype=np.int64)
    batch = np.asarray(batch, dtype=np.int64)

    deg = np.bincount(dst, minlength=N_NODES).astype(np.float64) + 1.0
    dinv = (1.0 / np.sqrt(deg)).astype(np.float32)

    # table row (256-wide) for node n in the chunked-AG table
    def grow(n):
        o = n // NPC
        r = n % NPC
        k = r // CH_ROWS
        rr = r % CH_ROWS
        return ((k * N_CORES + o) * P + rr % P) * CHM + rr // P

    # ---- layer-1 edge slots: dst-sharded, grouped by dst window ----
    owner = dst // NPC
    win = (dst % NPC) // P
    cnt = np.zeros((N_CORES, NT), dtype=np.int64)
    np.add.at(cnt, (owner, win), 1)
    tw = np.maximum((cnt.max(axis=0) + P - 1) // P, 1).astype(np.int64)
    ET1 = int(tw.sum())
    pad1 = (-ET1) % GB
    tw[NT - 1] += pad1
    ET1 += pad1
    col_base = np.zeros(NT, dtype=np.int64)
    col_base[1:] = np.cumsum(tw)[:-1]

    gsrc1 = np.zeros((N_CORES, P, ET1), dtype=np.int32)
    gdst1 = np.full((N_CORES, P, ET1), -1.0, dtype=np.float32)

    order = np.lexsort((dst, win, owner))
    so, sw = owner[order], win[order]
    ssrc, sdst = src[order], dst[order]
    grp = so * NT + sw
    gcount = np.zeros(N_CORES * NT, dtype=np.int64)
    np.add.at(gcount, grp, 1)
    starts = np.zeros(N_CORES * NT, dtype=np.int64)
    starts[1:] = np.cumsum(gcount)[:-1]
    pos = np.arange(len(order)) - starts[grp]
    col = col_base[sw] + pos // P
    row = pos % P
    gsrc1[so, row, col] = grow(ssrc).astype(np.int32)
    gdst1[so, row, col] = (sdst % NPC - sw * P).astype(np.float32)

    # ---- layer-2 slots: src-sharded (out-edges), grouped by graph window ----
    gofdst = batch[dst]
    owner2 = src // NPC
    gw = gofdst // P
    cnt2 = np.zeros((N_CORES, NGW), dtype=np.int64)
    np.add.at(cnt2, (owner2, gw), 1)
    tw2 = np.maximum((cnt2.max(axis=0) + P - 1) // P, 1).astype(np.int64)
    ET2 = int(tw2.sum())
    pad2 = (-ET2) % GB
    tw2[NGW - 1] += pad2
    ET2 += pad2
    col_base2 = np.zeros(NGW, dtype=np.int64)
    col_base2[1:] = np.cumsum(tw2)[:-1]

    gsrc2 = np.zeros((N_CORES, P, ET2), dtype=np.int32)
    gdst2 = np.full((N_CORES, P, ET2), -1.0, dtype=np.float32)
    gcf2 = np.zeros((N_CORES, P, ET2), dtype=np.float32)

    # local u-table row index of node j (owner-local): p*NT + w
    def urow(n):
        r = n % NPC
        return (r % P) * NT + r // P

    order2 = np.lexsort((gofdst, gw, owner2))
    so2, sw2 = owner2[order2], gw[order2]
    ssrc2, sdst2, sg2 = src[order2], dst[order2], gofdst[order2]
    grp2 = so2 * NGW + sw2
    gcount2 = np.zeros(N_CORES * NGW, dtype=np.int64)
    np.add.at(gcount2, grp2, 1)
    starts2 = np.zeros(N_CORES * NGW, dtype=np.int64)
    starts2[1:] = np.cumsum(gcount2)[:-1]
    pos2 = np.arange(len(order2)) - starts2[grp2]
    col2 = col_base2[sw2] + pos2 // P
    row2 = pos2 % P
    gsrc2[so2, row2, col2] = urow(ssrc2).astype(np.int32)
    gdst2[so2, row2, col2] = (sg2 - sw2 * P).astype(np.float32)
    gcf2[so2, row2, col2] = dinv[ssrc2] * dinv[sdst2]

    # ---- per-core tensors ----
    w1t = np.ascontiguousarray(
        W1.reshape(K_TILES, P, HID_CH).transpose(1, 0, 2)
    ).astype(bf16)                                        # [128, 6, 256]
    b1t = np.broadcast_to(b1[None, None, :], (P, 1, HID_CH)).astype(bf16)
    b1t = np.ascontiguousarray(b1t)
    iota = np.ascontiguousarray(np.broadcast_to(
        np.arange(P, dtype=np.float32)[None, None, :], (P, 1, P)
    )).astype(bf16)
    ident = np.eye(P, dtype=np.float32).astype(bf16)

    in_maps = []
    for c in range(N_CORES):
        lo = c * NPC
        xs = np.zeros((NPC_PAD, IN_CH), dtype=np.float32)
        xs[:NPC] = x[lo:lo + NPC]
        # xt[a, m, k, j] = xs[m*128 + j, k*128 + a]
        xt = np.ascontiguousarray(
            xs.reshape(NT, P, K_TILES, P).transpose(3, 0, 2, 1)
        ).astype(bf16)                                    # [128, 196, 6, 128]
        dc = np.zeros(NPC_PAD, dtype=np.float32)
        dc[:NPC] = dinv[lo:lo + NPC]
        dcol = np.ascontiguousarray(dc.reshape(NT, P).T[:, :, None])
        in_maps.append({
            "xt": xt,
            "w1t": w1t,
            "b1t": b1t,
            "iota": iota,
            "ident": ident,
            "dcol": dcol,
            "gsrc1": np.ascontiguousarray(gsrc1[c]),
            "gdst1": np.ascontiguousarray(gdst1[c])[:, :, None].astype(bf16),
            "gsrc2": np.ascontiguousarray(gsrc2[c]),
            "gdst2": np.ascontiguousarray(gdst2[c])[:, :, None].astype(bf16),
            "gcf2": np.ascontiguousarray(gcf2[c])[:, :, None],
        })

    meta = {"ET1": ET1, "tw": tw.tolist(), "ET2": ET2, "tw2": tw2.tolist()}

    counts = np.bincount(batch, minlength=NUM_GRAPHS).astype(np.int64)
    seg_starts = np.minimum(
        np.searchsorted(batch, np.arange(NUM_GRAPHS)), N_NODES - 1
    )
    aux = {"W2": W2, "b2": b2, "counts": counts, "seg_starts": seg_starts,
           "dinv": dinv}
    return meta, in_maps, aux


def _build(meta):
    import concourse.bass as bass
    import concourse.bacc as bacc
    import concourse.mybir as mybir
    import concourse.tile as tile

    ET1, tw = meta["ET1"], meta["tw"]
    ET2, tw2 = meta["ET2"], meta["tw2"]
    f32 = mybir.dt.float32
    bf16 = mybir.dt.bfloat16
    i32 = mybir.dt.int32

    nc = bacc.Bacc()
    xt = nc.declare_dram_parameter("xt", [P, NT, K_TILES, P], bf16, isOutput=False)
    w1t_d = nc.declare_dram_parameter("w1t", [P, K_TILES, HID_CH], bf16, isOutput=False)
    b1t_d = nc.declare_dram_parameter("b1t", [P, 1, HID_CH], bf16, isOutput=False)
    iota_d = nc.declare_dram_parameter("iota", [P, 1, P], bf16, isOutput=False)
    ident_d = nc.declare_dram_parameter("ident", [P, P], bf16, isOutput=False)
    dcol_d = nc.declare_dram_parameter("dcol", [P, NT, 1], f32, isOutput=False)
    gsrc1_d = nc.declare_dram_parameter("gsrc1", [P, ET1], i32, isOutput=False)
    gdst1_d = nc.declare_dram_parameter("gdst1", [P, ET1, 1], bf16, isOutput=False)
    gsrc2_d = nc.declare_dram_parameter("gsrc2", [P, ET2], i32, isOutput=False)
    gdst2_d = nc.declare_dram_parameter("gdst2", [P, ET2, 1], bf16, isOutput=False)
    gcf2_d = nc.declare_dram_parameter("gcf2", [P, ET2, 1], f32, isOutput=False)
    u_out = nc.declare_dram_parameter("u_out", [P, NT, HID_CH], bf16, isOutput=True)
    opart = nc.declare_dram_parameter("opart", [P, NGW, HID_CH], f32, isOutput=True)

    f8 = mybir.dt.float8e4
    h1l = nc.dram_tensor("h1l", [NCH, P, CHM, HID_CH], f8)
    h1full = nc.dram_tensor(
        "h1full", [NCH, N_CORES * P, CHM, HID_CH], f8, addr_space="Shared"
    )

    win_of_col1 = []
    for w in range(NT):
        win_of_col1 += [w] * tw[w]
    win_of_col2 = []
    for w in range(NGW):
        win_of_col2 += [w] * tw2[w]

    with tile.TileContext(nc) as tc:
        with (
            tc.tile_pool(name="cb", bufs=1) as cbuf,
            tc.tile_pool(name="sb", bufs=3) as sbuf,
            tc.tile_pool(name="xs", bufs=2) as xbuf,
            tc.tile_pool(name="st", bufs=2) as stg,
            tc.tile_pool(name="ps", bufs=2, space="PSUM") as psum,
            tc.tile_pool(name="ps1", bufs=2, space="PSUM") as psum1,
        ):
            # ---- constants ----
            w1_t = cbuf.tile([P, K_TILES, HID_CH], bf16)
            nc.sync.dma_start(out=w1_t[:], in_=w1t_d[:])
            b1_t = cbuf.tile([P, 1, HID_CH], bf16)
            nc.sync.dma_start(out=b1_t[:], in_=b1t_d[:])
            iota_t = cbuf.tile([P, 1, P], bf16)
            nc.sync.dma_start(out=iota_t[:], in_=iota_d[:])
            ident_t = cbuf.tile([P, P], bf16)
            nc.sync.dma_start(out=ident_t[:], in_=ident_d[:])
            dcol_t = cbuf.tile([P, NT, 1], f32)
            nc.sync.dma_start(out=dcol_t[:], in_=dcol_d[:])
            gsrc1_t = cbuf.tile([P, ET1], i32)
            nc.sync.dma_start(out=gsrc1_t[:], in_=gsrc1_d[:])
            gdst1_t = cbuf.tile([P, ET1, 1], bf16)
            nc.sync.dma_start(out=gdst1_t[:], in_=gdst1_d[:])
            gsrc2_t = cbuf.tile([P, ET2], i32)
            nc.sync.dma_start(out=gsrc2_t[:], in_=gsrc2_d[:])
            gdst2_t = cbuf.tile([P, ET2, 1], bf16)
            nc.sync.dma_start(out=gdst2_t[:], in_=gdst2_d[:])
            gcf2_t = cbuf.tile([P, ET2, 1], f32)
            nc.sync.dma_start(out=gcf2_t[:], in_=gcf2_d[:])
            hloc = cbuf.tile([P, NT, HID_CH], bf16)     # h~ local (SBUF)

            # ---- stage 1: h~ = dinv * (x @ W1) + chunked AllGather ----
            n_mb = (NT + MB - 1) // MB
            next_chunk = 0
            acc1 = None
            for blk in range(n_mb):
                m0 = blk * MB
                mcnt = min(MB, NT - m0)
                xts = xbuf.tile([P, MB, K_TILES, P], bf16, tag="xts")
                nc.sync.dma_start(
                    out=xts[:, 0:mcnt, :, :], in_=xt[:, m0:m0 + mcnt, :, :],
                )
                for mm in range(mcnt):
                    m = m0 + mm
                    sl = m % WB
                    if sl == 0:
                        acc1 = psum1.tile([P, WB, HID_CH], f32, tag="acc1")
                    for k in range(K_TILES):
                        nc.tensor.matmul(
                            acc1[:, sl, :],
                            lhsT=xts[:, mm, k, :],
                            rhs=w1_t[:, k, :],
                            start=(k == 0),
                            stop=(k == K_TILES - 1),
                        )
                    if sl == WB - 1 or m == NT - 1:
                        w0 = m - sl
                        nc.vector.tensor_tensor(
                            out=hloc[:, w0:m + 1, :],
                            in0=acc1[:, 0:sl + 1, :],
                            in1=dcol_t[:, w0:m + 1, :].to_broadcast(
                                [P, sl + 1, HID_CH]
                            ),
                            op=mybir.AluOpType.mult,
                        )
                m_done = m0 + mcnt
                # stream h1l (SWDGE cast bf16 -> fp8) per block, split at
                # chunk boundaries so each AllGather fires without waiting
                # on a 49-tile cast
                mm0 = m0
                while mm0 < m_done:
                    k = mm0 // CHM
                    hi = min(m_done, (k + 1) * CHM)
                    nc.gpsimd.dma_start(
                        out=h1l[k][:, mm0 - k * CHM:hi - k * CHM, :],
                        in_=hloc[:, mm0:hi, :],
                    )
                    mm0 = hi
                while next_chunk < NCH and m_done >= (next_chunk + 1) * CHM:
                    k = next_chunk
                    nc.gpsimd.collective_compute(
                        "AllGather", mybir.AluOpType.bypass,
                        ins=[h1l[k]], outs=[h1full[k]],
                        replica_groups=[list(range(N_CORES))],
                    )
                    next_chunk += 1

            # ---- layer 1: aggregate -> u table ----
            table1 = h1full[:].rearrange("k r m c -> (k r m) c")
            ustage = None
            accw = None
            for b in range(ET1 // GB):
                c0 = b * GB
                msg = sbuf.tile([P, GB, HID_CH], bf16, tag="msg")
                for t in range(GB):
                    nc.gpsimd.indirect_dma_start(
                        out=msg[:, t, :], out_offset=None,
                        in_=table1,
                        in_offset=bass.IndirectOffsetOnAxis(
                            ap=gsrc1_t[:, c0 + t:c0 + t + 1], axis=0
                        ),
                    )
                pt = sbuf.tile([P, GB, P], bf16, tag="pt")
                nc.vector.tensor_tensor(
                    out=pt[:],
                    in0=gdst1_t[:, c0:c0 + GB, :].to_broadcast([P, GB, P]),
                    in1=iota_t[:].to_broadcast([P, GB, P]),
                    op=mybir.AluOpType.is_equal,
                )
                for t in range(GB):
                    c = c0 + t
                    w = win_of_col1[c]
                    first = (c == 0) or (win_of_col1[c - 1] != w)
                    last = (c == ET1 - 1) or (win_of_col1[c + 1] != w)
                    sl = w % WB
                    if first:
                        if sl == 0:
                            accw = psum.tile([P, WB, HID_CH], f32, tag="accw")
                        nc.tensor.matmul(
                            accw[:, sl, :], lhsT=ident_t[:], rhs=hloc[:, w, :],
                            start=True, stop=False,
                        )
                    nc.tensor.matmul(
                        accw[:, sl, :], lhsT=pt[:, t, :], rhs=msg[:, t, :],
                        start=False, stop=last,
                    )
                    if last and (sl == WB - 1 or w == NT - 1):
                        # u = relu(dinv*acc + b1) for WB windows
                        w0 = w - sl
                        us = w % MB
                        tm = sbuf.tile([P, WB, HID_CH], bf16, tag="tm")
                        nc.vector.tensor_tensor(
                            out=tm[:, 0:sl + 1, :],
                            in0=accw[:, 0:sl + 1, :],
                            in1=dcol_t[:, w0:w + 1, :].to_broadcast(
                                [P, sl + 1, HID_CH]
                            ),
                            op=mybir.AluOpType.mult,
                        )
                        nc.vector.tensor_tensor(
                            out=tm[:, 0:sl + 1, :],
                            in0=tm[:, 0:sl + 1, :],
                            in1=b1_t[:].to_broadcast([P, sl + 1, HID_CH]),
                            op=mybir.AluOpType.add,
                        )
                        if ustage is None:
                            ustage = stg.tile([P, MB, HID_CH], bf16, tag="ustage")
                        nc.scalar.activation(
                            out=ustage[:, us - sl:us + 1, :],
                            in_=tm[:, 0:sl + 1, :],
                            func=mybir.ActivationFunctionType.Relu,
                        )
                        if us == MB - 1 or w == NT - 1:
                            nc.sync.dma_start(
                                out=u_out[:, w - us:w + 1, :],
                                in_=ustage[:, 0:us + 1, :],
                            )
                            ustage = None

            # ---- layer 2 + pooling partials ----
            table2 = u_out[:].rearrange("p w c -> (p w) c")
            ostage = None
            accg = None
            for b in range(ET2 // GB):
                c0 = b * GB
                msg2 = sbuf.tile([P, GB, HID_CH], bf16, tag="msg")
                for t in range(GB):
                    nc.gpsimd.indirect_dma_start(
                        out=msg2[:, t, :], out_offset=None,
                        in_=table2,
                        in_offset=bass.IndirectOffsetOnAxis(
                            ap=gsrc2_t[:, c0 + t:c0 + t + 1], axis=0
                        ),
                    )
                pt2 = sbuf.tile([P, GB, P], bf16, tag="pt")
                nc.vector.tensor_tensor(
                    out=pt2[:],
                    in0=gdst2_t[:, c0:c0 + GB, :].to_broadcast([P, GB, P]),
                    in1=iota_t[:].to_broadcast([P, GB, P]),
                    op=mybir.AluOpType.is_equal,
                )
                nc.vector.tensor_tensor(
                    out=pt2[:],
                    in0=pt2[:],
                    in1=gcf2_t[:, c0:c0 + GB, :].to_broadcast([P, GB, P]),
                    op=mybir.AluOpType.mult,
                )
                for t in range(GB):
                    c = c0 + t
                    w = win_of_col2[c]
                    first = (c == 0) or (win_of_col2[c - 1] != w)
                    last = (c == ET2 - 1) or (win_of_col2[c + 1] != w)
                    sl = w % WB
                    if first and sl == 0:
                        accg = psum.tile([P, WB, HID_CH], f32, tag="accw")
                    nc.tensor.matmul(
                        accg[:, sl, :], lhsT=pt2[:, t, :], rhs=msg2[:, t, :],
                        start=first, stop=last,
                    )
                    if last and (sl == WB - 1 or w == NGW - 1):
                        us = w % MB
                        if ostage is None:
                            ostage = stg.tile([P, MB, HID_CH], f32, tag="ostage")
                        nc.vector.tensor_copy(
                            out=ostage[:, us - sl:us + 1, :],
                            in_=accg[:, 0:sl + 1, :],
                        )
                        if us == MB - 1 or w == NGW - 1:
                            nc.sync.dma_start(
                                out=opart[:, w - us:w + 1, :],
                                in_=ostage[:, 0:us + 1, :],
                            )
                            ostage = None

    nc.finalize()
    return nc


def _host_epilogue(results, aux):
    counts = aux["counts"]
    dinv = aux["dinv"]
    pooled_pre = np.zeros((NG_PAD, HID_CH), dtype=np.float32)
    for c in range(N_CORES):
        op = np.asarray(results[c]["opart"], dtype=np.float32)
        pooled_pre += op.reshape(P, NGW, HID_CH).transpose(1, 0, 2).reshape(
            NG_PAD, HID_CH
        )
    pooled_pre = pooled_pre[:NUM_GRAPHS]
    rs = []
    for c in range(N_CORES):
        u = np.asarray(results[c]["u_out"]).astype(np.float32)
        rs.append(
            u.reshape(P, NT, HID_CH).transpose(1, 0, 2).reshape(
                NPC_PAD, HID_CH
            )[:NPC]
        )
    r = np.concatenate(rs, axis=0)
    v = r * (dinv * dinv)[:, None]
    sums_self = np.add.reduceat(v, aux["seg_starts"], axis=0)
    sums_self[counts == 0] = 0.0
    pooled_pre = pooled_pre + sums_self
    pooled = pooled_pre / np.maximum(counts, 1)[:, None]
    out = pooled.astype(np.float32) @ aux["W2"] + aux["b2"]
    out[counts == 0] = 0.0
    return out.astype(np.float32)


def kernel(**inputs):
    from concourse.bass_utils import run_bass_kernel_spmd

    meta, in_maps, aux = _prep(
        inputs["x"], inputs["W1"], inputs["b1"], inputs["W2"], inputs["b2"],
        inputs["edge_index"], inputs["batch"],
    )
    nc = _build(meta)
    res = run_bass_kernel_spmd(nc, in_maps, list(range(N_CORES)))
    return _host_epilogue(res.results, aux)


# revision 12
# speedup vs baseline: 1.0298x; 1.0298x over previous
"""Trainium2 Bass kernel for DependencyGNN (2-layer GCN + global mean pool).

Distribution (8 cores, SPMD):
  - Nodes sharded contiguously (25000/core, padded to 25088 = 196*128).
  - Stage 1: h~ = dinv * (x_shard @ W1) in bf16, kept in SBUF and written to
    DRAM in 4 chunks; each chunk is AllGathered (bf16) while stage 1 continues.
  - Layer 1 (dst-sharded edges): batched indirect gathers of h~[src] from the
    AG table; one-hot (is_equal) selection matmuls accumulate per 128-dst
    window into PSUM; self-loop added via identity matmul; epilogue
    relu(dinv*acc + b1) -> u table (bf16, local DRAM + kernel output).
  - Layer 2 + mean-pool fused: out-edge (src-local) gathers of u, coefficient
    dinv[dst]*dinv[src], one-hot matmul onto per-graph windows -> per-core
    partial pooled sums (f32 output). No second collective.
  - Host: self-loop pooling terms from u, sum partials, /counts, @W2 + b2.
"""
import numpy as np

N_NODES = 200000
N_EDGES = 400000
IN_CH, HID_CH, OUT_CH = 768, 256, 256
NUM_GRAPHS = 8000
N_CORES = 8
P = 128
NPC = N_NODES // N_CORES            # 25000
NT = (NPC + P - 1) // P             # 196 m-tiles per core
NPC_PAD = NT * P                    # 25088
K_TILES = IN_CH // P                # 6
NCH = 4                             # AllGather chunks
CHM = NT // NCH                     # 49 m-tiles per chunk
CH_ROWS = CHM * P                   # 6272 rows per chunk
NGW = (NUM_GRAPHS + P - 1) // P     # 63 graph windows (8064 padded)
NG_PAD = NGW * P
GB = 16                             # gather batch (tiles per indirect DMA)
MB = 8                              # stage-1 m-tile DMA batch / write batch
WB = 4                              # epilogue window batch (PSUM group)


def _bf16():
    import concourse.mybir as mybir
    return mybir.dt.np(mybir.dt.bfloat16)


def _prep(x, W1, b1, W2, b2, edge_index, batch):
    bf16 = _bf16()
    x = np.asarray(x, dtype=np.float32)
    W1 = np.asarray(W1, dtype=np.float32)
    b1 = np.asarray(b1, dtype=np.float32)
    W2 = np.asarray(W2, dtype=np.float32)
    b2 = np.asarray(b2, dtype=np.float32)
    src = np.asarray(edge_index[0], dtype=np.int64)
    dst = np.asarray(edge_index[1], dtype=np.int64)
    batch = np.asarray(batch, dtype=np.int64)

    deg = np.bincount(dst, minlength=N_NODES).astype(np.float64) + 1.0
    dinv = (1.0 / np.sqrt(deg)).astype(np.float32)

    # table row (256-wide) for node n in the chunked-AG table
    def grow(n):
        o = n // NPC
        r = n % NPC
        k = r // CH_ROWS
        rr = r % CH_ROWS
        return ((k * N_CORES + o) * P + rr % P) * CHM + rr // P

    # ---- layer-1 edge slots: dst-sharded, grouped by dst window ----
    owner = dst // NPC
    win = (dst % NPC) // P
    cnt = np.zeros((N_CORES, NT), dtype=np.int64)
    np.add.at(cnt, (owner, win), 1)
    tw = np.maximum((cnt.max(axis=0) + P - 1) // P, 1).astype(np.int64)
    ET1 = int(tw.sum())
    pad1 = (-ET1) % GB
    tw[NT - 1] += pad1
    ET1 += pad1
    col_base = np.zeros(NT, dtype=np.int64)
    col_base[1:] = np.cumsum(tw)[:-1]

    gsrc1 = np.zeros((N_CORES, P, ET1), dtype=np.int32)
    gdst1 = np.full((N_CORES, P, ET1), -1.0, dtype=np.float32)

    order = np.lexsort((dst, win, owner))
    so, sw = owner[order], win[order]
    ssrc, sdst = src[order], dst[order]
    grp = so * NT + sw
    gcount = np.zeros(N_CORES * NT, dtype=np.int64)
    np.add.at(gcount, grp, 1)
    starts = np.zeros(N_CORES * NT, dtype=np.int64)
    starts[1:] = np.cumsum(gcount)[:-1]
    pos = np.arange(len(order)) - starts[grp]
    col = col_base[sw] + pos // P
    row = pos % P
    gsrc1[so, row, col] = grow(ssrc).astype(np.int32)
    gdst1[so, row, col] = (sdst % NPC - sw * P).astype(np.float32)

    # ---- layer-2 slots: src-sharded (out-edges), grouped by graph window ----
    gofdst = batch[dst]
    owner2 = src // NPC
    gw = gofdst // P
    cnt2 = np.zeros((N_CORES, NGW), dtype=np.int64)
    np.add.at(cnt2, (owner2, gw), 1)
    tw2 = np.maximum((cnt2.max(axis=0) + P - 1) // P, 1).astype(np.int64)
    ET2 = int(tw2.sum())
    pad2 = (-ET2) % GB
    tw2[NGW - 1] += pad2
    ET2 += pad2
    col_base2 = np.zeros(NGW, dtype=np.int64)
    col_base2[1:] = np.cumsum(tw2)[:-1]

    gsrc2 = np.zeros((N_CORES, P, ET2), dtype=np.int32)
    gdst2 = np.full((N_CORES, P, ET2), -1.0, dtype=np.float32)
    gcf2 = np.zeros((N_CORES, P, ET2), dtype=np.float32)

    # local u-table row index of node j (owner-local): p*NT + w
    def urow(n):
        r = n % NPC
        return (r % P) * NT + r // P

    order2 = np.lexsort((gofdst, gw, owner2))
    so2, sw2 = owner2[order2], gw[order2]
    ssrc2, sdst2, sg2 = src[order2], dst[order2], gofdst[order2]
    grp2 = so2 * NGW + sw2
    gcount2 = np.zeros(N_CORES * NGW, dtype=np.int64)
    np.add.at(gcount2, grp2, 1)
    starts2 = np.zeros(N_CORES * NGW, dtype=np.int64)
    starts2[1:] = np.cumsum(gcount2)[:-1]
    pos2 = np.arange(len(order2)) - starts2[grp2]
    col2 = col_base2[sw2] + pos2 // P
    row2 = pos2 % P
    gsrc2[so2, row2, col2] = urow(ssrc2).astype(np.int32)
    gdst2[so2, row2, col2] = (sg2 - sw2 * P).astype(np.float32)
    gcf2[so2, row2, col2] = dinv[ssrc2] * dinv[sdst2]

    # ---- per-core tensors ----
    w1t = np.ascontiguousarray(
        W1.reshape(K_TILES, P, HID_CH).transpose(1, 0, 2)
    ).astype(bf16)                                        # [128, 6, 256]
    b1t = np.broadcast_to(b1[None, None, :], (P, 1, HID_CH)).astype(bf16)
    b1t = np.ascontiguousarray(b1t)
    iota = np.ascontiguousarray(np.broadcast_to(
        np.arange(P, dtype=np.float32)[None, None, :], (P, 1, P)
    )).astype(bf16)
    ident = np.eye(P, dtype=np.float32).astype(bf16)

    in_maps = []
    for c in range(N_CORES):
        lo = c * NPC
        xs = np.zeros((NPC_PAD, IN_CH), dtype=np.float32)
        xs[:NPC] = x[lo:lo + NPC]
        # xt[a, m, k, j] = xs[m*128 + j, k*128 + a]
        xt = np.ascontiguousarray(
            xs.reshape(NT, P, K_TILES, P).transpose(3, 0, 2, 1)
        ).astype(bf16)                                    # [128, 196, 6, 128]
        dc = np.zeros(NPC_PAD, dtype=np.float32)
        dc[:NPC] = dinv[lo:lo + NPC]
        dcol = np.ascontiguousarray(dc.reshape(NT, P).T[:, :, None])
        in_maps.append({
            "xt": xt,
            "w1t": w1t,
            "b1t": b1t,
            "iota": iota,
            "ident": ident,
            "dcol": dcol,
            "gsrc1": np.ascontiguousarray(gsrc1[c]),
            "gdst1": np.ascontiguousarray(gdst1[c])[:, :, None].astype(bf16),
            "gsrc2": np.ascontiguousarray(gsrc2[c]),
            "gdst2": np.ascontiguousarray(gdst2[c])[:, :, None].astype(bf16),
            "gcf2": np.ascontiguousarray(gcf2[c])[:, :, None],
        })

    meta = {"ET1": ET1, "tw": tw.tolist(), "ET2": ET2, "tw2": tw2.tolist()}

    counts = np.bincount(batch, minlength=NUM_GRAPHS).astype(np.int64)
    seg_starts = np.minimum(
        np.searchsorted(batch, np.arange(NUM_GRAPHS)), N_NODES - 1
    )
    aux = {"W2": W2, "b2": b2, "counts": counts, "seg_starts": seg_starts,
           "dinv": dinv}
    return meta, in_maps, aux


def _build(meta):
    import concourse.bass as bass
    import concourse.bacc as bacc
    import concourse.mybir as mybir
    import concourse.tile as tile

    ET1, tw = meta["ET1"], meta["tw"]
    ET2, tw2 = meta["ET2"], meta["tw2"]
    f32 = mybir.dt.float32
    bf16 = mybir.dt.bfloat16
    i32 = mybir.dt.int32

    nc = bacc.Bacc()
    xt = nc.declare_dram_parameter("xt", [P, NT, K_TILES, P], bf16, isOutput=False)
    w1t_d = nc.declare_dram_parameter("w1t", [P, K_TILES, HID_CH], bf16, isOutput=False)
    b1t_d = nc.declare_dram_parameter("b1t", [P, 1, HID_CH], bf16, isOutput=False)
    iota_d = nc.declare_dram_parameter("iota", [P, 1, P], bf16, isOutput=False)
    ident_d = nc.declare_dram_parameter("ident", [P, P], bf16, isOutput=False)
    dcol_d = nc.declare_dram_parameter("dcol", [P, NT, 1], f32, isOutput=False)
    gsrc1_d = nc.declare_dram_parameter("gsrc1", [P, ET1], i32, isOutput=False)
    gdst1_d = nc.declare_dram_parameter("gdst1", [P, ET1, 1], bf16, isOutput=False)
    gsrc2_d = nc.declare_dram_parameter("gsrc2", [P, ET2], i32, isOutput=False)
    gdst2_d = nc.declare_dram_parameter("gdst2", [P, ET2, 1], bf16, isOutput=False)
    gcf2_d = nc.declare_dram_parameter("gcf2", [P, ET2, 1], f32, isOutput=False)
    u_out = nc.declare_dram_parameter("u_out", [P, NT, HID_CH], bf16, isOutput=True)
    opart = nc.declare_dram_parameter("opart", [P, NGW, HID_CH], f32, isOutput=True)

    f8 = mybir.dt.float8e4
    h1l = nc.dram_tensor("h1l", [NCH, P, CHM, HID_CH], f8)
    h1full = nc.dram_tensor(
        "h1full", [NCH, N_CORES * P, CHM, HID_CH], f8, addr_space="Shared"
    )

    win_of_col1 = []
    for w in range(NT):
        win_of_col1 += [w] * tw[w]
    win_of_col2 = []
    for w in range(NGW):
        win_of_col2 += [w] * tw2[w]

    with tile.TileContext(nc) as tc:
        with (
            tc.tile_pool(name="cb", bufs=1) as cbuf,
            tc.tile_pool(name="sb", bufs=3) as sbuf,
            tc.tile_pool(name="xs", bufs=2) as xbuf,
            tc.tile_pool(name="st", bufs=2) as stg,
            tc.tile_pool(name="ps", bufs=2, space="PSUM") as psum,
            tc.tile_pool(name="ps1", bufs=2, space="PSUM") as psum1,
        ):
            # ---- constants ----
            w1_t = cbuf.tile([P, K_TILES, HID_CH], bf16)
            nc.sync.dma_start(out=w1_t[:], in_=w1t_d[:])
            b1_t = cbuf.tile([P, 1, HID_CH], bf16)
            nc.sync.dma_start(out=b1_t[:], in_=b1t_d[:])
            iota_t = cbuf.tile([P, 1, P], bf16)
            nc.sync.dma_start(out=iota_t[:], in_=iota_d[:])
            ident_t = cbuf.tile([P, P], bf16)
            nc.sync.dma_start(out=ident_t[:], in_=ident_d[:])
            dcol_t = cbuf.tile([P, NT, 1], f32)
            nc.sync.dma_start(out=dcol_t[:], in_=dcol_d[:])
            gsrc1_t = cbuf.tile([P, ET1], i32)
            nc.sync.dma_start(out=gsrc1_t[:], in_=gsrc1_d[:])
            gdst1_t = cbuf.tile([P, ET1, 1], bf16)
            nc.sync.dma_start(out=gdst1_t[:], in_=gdst1_d[:])
            gsrc2_t = cbuf.tile([P, ET2], i32)
            nc.sync.dma_start(out=gsrc2_t[:], in_=gsrc2_d[:])
            gdst2_t = cbuf.tile([P, ET2, 1], bf16)
            nc.sync.dma_start(out=gdst2_t[:], in_=gdst2_d[:])
            gcf2_t = cbuf.tile([P, ET2, 1], f32)
            nc.sync.dma_start(out=gcf2_t[:], in_=gcf2_d[:])
            hloc = cbuf.tile([P, NT, HID_CH], bf16)     # h~ local (SBUF)

            # ---- stage 1: h~ = dinv * (x @ W1) + chunked AllGather ----
            n_mb = (NT + MB - 1) // MB
            next_chunk = 0
            acc1 = None
            for blk in range(n_mb):
                m0 = blk * MB
                mcnt = min(MB, NT - m0)
                xts = xbuf.tile([P, MB, K_TILES, P], bf16, tag="xts")
                nc.sync.dma_start(
                    out=xts[:, 0:mcnt, :, :], in_=xt[:, m0:m0 + mcnt, :, :],
                )
                for mm in range(mcnt):
                    m = m0 + mm
                    sl = m % WB
                    if sl == 0:
                        acc1 = psum1.tile([P, WB, HID_CH], f32, tag="acc1")
                    for k in range(K_TILES):
                        nc.tensor.matmul(
                            acc1[:, sl, :],
                            lhsT=xts[:, mm, k, :],
                            rhs=w1_t[:, k, :],
                            start=(k == 0),
                            stop=(k == K_TILES - 1),
                        )
                    if sl == WB - 1 or m == NT - 1:
                        w0 = m - sl
                        nc.vector.tensor_tensor(
                            out=hloc[:, w0:m + 1, :],
                            in0=acc1[:, 0:sl + 1, :],
                            in1=dcol_t[:, w0:m + 1, :].to_broadcast(
                                [P, sl + 1, HID_CH]
                            ),
                            op=mybir.AluOpType.mult,
                        )
                m_done = m0 + mcnt
                while next_chunk < NCH and m_done >= (next_chunk + 1) * CHM:
                    k = next_chunk
                    # SWDGE cast bf16 -> fp8 during the write
                    nc.gpsimd.dma_start(
                        out=h1l[k], in_=hloc[:, k * CHM:(k + 1) * CHM, :],
                    )
                    nc.gpsimd.collective_compute(
                        "AllGather", mybir.AluOpType.bypass,
                        ins=[h1l[k]], outs=[h1full[k]],
                        replica_groups=[list(range(N_CORES))],
                    )
                    next_chunk += 1

            # ---- layer 1: aggregate -> u table ----
            table1 = h1full[:].rearrange("k r m c -> (k r m) c")
            ustage = None
            accw = None
            for b in range(ET1 // GB):
                c0 = b * GB
                msg = sbuf.tile([P, GB, HID_CH], bf16, tag="msg")
                for t in range(GB):
                    nc.gpsimd.indirect_dma_start(
                        out=msg[:, t, :], out_offset=None,
                        in_=table1,
                        in_offset=bass.IndirectOffsetOnAxis(
                            ap=gsrc1_t[:, c0 + t:c0 + t + 1], axis=0
                        ),
                    )
                pt = sbuf.tile([P, GB, P], bf16, tag="pt")
                nc.vector.tensor_tensor(
                    out=pt[:],
                    in0=gdst1_t[:, c0:c0 + GB, :].to_broadcast([P, GB, P]),
                    in1=iota_t[:].to_broadcast([P, GB, P]),
                    op=mybir.AluOpType.is_equal,
                )
                for t in range(GB):
                    c = c0 + t
                    w = win_of_col1[c]
                    first = (c == 0) or (win_of_col1[c - 1] != w)
                    last = (c == ET1 - 1) or (win_of_col1[c + 1] != w)
                    sl = w % WB
                    if first:
                        if sl == 0:
                            accw = psum.tile([P, WB, HID_CH], f32, tag="accw")
                        nc.tensor.matmul(
                            accw[:, sl, :], lhsT=ident_t[:], rhs=hloc[:, w, :],
                            start=True, stop=False,
                        )
                    nc.tensor.matmul(
                        accw[:, sl, :], lhsT=pt[:, t, :], rhs=msg[:, t, :],
                        start=False, stop=last,
                    )
                    if last and (sl == WB - 1 or w == NT - 1):
                        # u = relu(dinv*acc + b1) for WB windows
                        w0 = w - sl
                        us = w % MB
                        tm = sbuf.tile([P, WB, HID_CH], bf16, tag="tm")
                        nc.vector.tensor_tensor(
                            out=tm[:, 0:sl + 1, :],
                            in0=accw[:, 0:sl + 1, :],
                            in1=dcol_t[:, w0:w + 1, :].to_broadcast(
                                [P, sl + 1, HID_CH]
                            ),
                            op=mybir.AluOpType.mult,
                        )
                        nc.vector.tensor_tensor(
                            out=tm[:, 0:sl + 1, :],
                            in0=tm[:, 0:sl + 1, :],
                            in1=b1_t[:].to_broadcast([P, sl + 1, HID_CH]),
                            op=mybir.AluOpType.add,
                        )
                        if ustage is None:
                            ustage = stg.tile([P, MB, HID_CH], bf16, tag="ustage")
                        nc.scalar.activation(
                            out=ustage[:, us - sl:us + 1, :],
                            in_=tm[:, 0:sl + 1, :],
                            func=mybir.ActivationFunctionType.Relu,
                        )
                        if us == MB - 1 or w == NT - 1:
                            nc.sync.dma_start(
                                out=u_out[:, w - us:w + 1, :],
                                in_=ustage[:, 0:us + 1, :],
                            )
                            ustage = None

            # ---- layer 2 + pooling partials ----
            table2 = u_out[:].rearrange("p w c -> (p w) c")
            ostage = None
            accg = None
            for b in range(ET2 // GB):
                c0 = b * GB
                msg2 = sbuf.tile([P, GB, HID_CH], bf16, tag="msg")
                for t in range(GB):
                    nc.gpsimd.indirect_dma_start(
                        out=msg2[:, t, :], out_offset=None,
                        in_=table2,
                        in_offset=bass.IndirectOffsetOnAxis(
                            ap=gsrc2_t[:, c0 + t:c0 + t + 1], axis=0
                        ),
                    )
                pt2 = sbuf.tile([P, GB, P], bf16, tag="pt")
                nc.vector.tensor_tensor(
                    out=pt2[:],
                    in0=gdst2_t[:, c0:c0 + GB, :].to_broadcast([P, GB, P]),
                    in1=iota_t[:].to_broadcast([P, GB, P]),
                    op=mybir.AluOpType.is_equal,
                )
                nc.vector.tensor_tensor(
                    out=pt2[:],
                    in0=pt2[:],
                    in1=gcf2_t[:, c0:c0 + GB, :].to_broadcast([P, GB, P]),
                    op=mybir.AluOpType.mult,
                )
                for t in range(GB):
                    c = c0 + t
                    w = win_of_col2[c]
                    first = (c == 0) or (win_of_col2[c - 1] != w)
                    last = (c == ET2 - 1) or (win_of_col2[c + 1] != w)
                    sl = w % WB
                    if first and sl == 0:
                        accg = psum.tile([P, WB, HID_CH], f32, tag="accw")
                    nc.tensor.matmul(
                        accg[:, sl, :], lhsT=pt2[:, t, :], rhs=msg2[:, t, :],
                        start=first, stop=last,
                    )
                    if last and (sl == WB - 1 or w == NGW - 1):
                        us = w % MB
                        if ostage is None:
                            ostage = stg.tile([P, MB, HID_CH], f32, tag="ostage")
                        nc.vector.tensor_copy(
                            out=ostage[:, us - sl:us + 1, :],
                            in_=accg[:, 0:sl + 1, :],
                        )
                        if us == MB - 1 or w == NGW - 1:
                            nc.sync.dma_start(
                                out=opart[:, w - us:w + 1, :],
                                in_=ostage[:, 0:us + 1, :],
                            )
                            ostage = None

    nc.finalize()
    return nc


def _host_epilogue(results, aux):
    counts = aux["counts"]
    dinv = aux["dinv"]
    pooled_pre = np.zeros((NG_PAD, HID_CH), dtype=np.float32)
    for c in range(N_CORES):
        op = np.asarray(results[c]["opart"], dtype=np.float32)
        pooled_pre += op.reshape(P, NGW, HID_CH).transpose(1, 0, 2).reshape(
            NG_PAD, HID_CH
        )
    pooled_pre = pooled_pre[:NUM_GRAPHS]
    rs = []
    for c in range(N_CORES):
        u = np.asarray(results[c]["u_out"]).astype(np.float32)
        rs.append(
            u.reshape(P, NT, HID_CH).transpose(1, 0, 2).reshape(
                NPC_PAD, HID_CH
            )[:NPC]
        )
    r = np.concatenate(rs, axis=0)
    v = r * (dinv * dinv)[:, None]
    sums_self = np.add.reduceat(v, aux["seg_starts"], axis=0)
    sums_self[counts == 0] = 0.0
    pooled_pre = pooled_pre + sums_self
    pooled = pooled_pre / np.maximum(counts, 1)[:, None]
    out = pooled.astype(np.float32) @ aux["W2"] + aux["b2"]
    out[counts == 0] = 0.0
    return out.astype(np.float32)


def kernel(**inputs):
    from concourse.bass_utils import run_bass_kernel_spmd

    meta, in_maps, aux = _prep(
        inputs["x"], inputs["W1"], inputs["b1"], inputs["W2"], inputs["b2"],
        inputs["edge_index"], inputs["batch"],
    )
    nc = _build(meta)
    res = run_bass_kernel_spmd(nc, in_maps, list(range(N_CORES)))
    return _host_epilogue(res.results, aux)


# revision 20
# speedup vs baseline: 1.0320x; 1.0021x over previous
"""Trainium2 Bass kernel for DependencyGNN (2-layer GCN + global mean pool).

Distribution (8 cores, SPMD):
  - Nodes sharded contiguously (25000/core, padded to 25088 = 196*128).
  - Stage 1: h~ = dinv * (x_shard @ W1) in bf16, kept in SBUF and written to
    DRAM in 4 chunks; each chunk is AllGathered (bf16) while stage 1 continues.
  - Layer 1 (dst-sharded edges): batched indirect gathers of h~[src] from the
    AG table; one-hot (is_equal) selection matmuls accumulate per 128-dst
    window into PSUM; self-loop added via identity matmul; epilogue
    relu(dinv*acc + b1) -> u table (bf16, local DRAM + kernel output).
  - Layer 2 + mean-pool fused: out-edge (src-local) gathers of u, coefficient
    dinv[dst]*dinv[src], one-hot matmul onto per-graph windows -> per-core
    partial pooled sums (f32 output). No second collective.
  - Host: self-loop pooling terms from u, sum partials, /counts, @W2 + b2.
"""
import numpy as np

N_NODES = 200000
N_EDGES = 400000
IN_CH, HID_CH, OUT_CH = 768, 256, 256
NUM_GRAPHS = 8000
N_CORES = 8
P = 128
NPC = N_NODES // N_CORES            # 25000
NT = (NPC + P - 1) // P             # 196 m-tiles per core
NPC_PAD = NT * P                    # 25088
K_TILES = IN_CH // P                # 6
NCH = 4                             # AllGather chunks
CHM = NT // NCH                     # 49 m-tiles per chunk
CH_ROWS = CHM * P                   # 6272 rows per chunk
NGW = (NUM_GRAPHS + P - 1) // P     # 63 graph windows (8064 padded)
NG_PAD = NGW * P
GB = 16                             # gather batch (tiles per indirect DMA)
MB = 8                              # stage-1 m-tile DMA batch / write batch
WB = 4                              # epilogue window batch (PSUM group)


def _bf16():
    import concourse.mybir as mybir
    return mybir.dt.np(mybir.dt.bfloat16)


def _prep(x, W1, b1, W2, b2, edge_index, batch):
    bf16 = _bf16()
    x = np.asarray(x, dtype=np.float32)
    W1 = np.asarray(W1, dtype=np.float32)
    b1 = np.asarray(b1, dtype=np.float32)
    W2 = np.asarray(W2, dtype=np.float32)
    b2 = np.asarray(b2, dtype=np.float32)
    src = np.asarray(edge_index[0], dtype=np.int64)
    dst = np.asarray(edge_index[1], dtype=np.int64)
    batch = np.asarray(batch, dtype=np.int64)

    deg_in = np.bincount(dst, minlength=N_NODES)
    deg = deg_in.astype(np.float64) + 1.0
    dinv = (1.0 / np.sqrt(deg)).astype(np.float32)

    # ---- per-core node permutation: balance in-edge counts per 128-node
    # window so tw[w] is 2 instead of 3 (fewer padded gather tiles) ----
    NB = NT - 1                       # 195 full bins; bin NT-1 takes the tail
    npos = np.zeros(N_NODES, dtype=np.int64)   # node -> permuted local row
    for c in range(N_CORES):
        lo = c * NPC
        d = deg_in[lo:lo + NPC].astype(np.int64)
        order_desc = np.argsort(-d, kind="stable")
        heavy = order_desc[:NPC - NB * P]      # 40 heaviest -> last window
        rest = order_desc[NPC - NB * P:]       # 24960 = 195*128
        # serpentine deal of degree-sorted rest into NB bins
        rounds = rest.reshape(P, NB)
        cols = np.arange(NB)
        binidx = np.empty((P, NB), dtype=np.int64)
        for r in range(P):
            binidx[r] = cols if r % 2 == 0 else cols[::-1]
        bins = np.full((NB, P), -1, dtype=np.int64)
        for r in range(P):
            bins[binidx[r], r] = rounds[r]
        sums = d[bins].sum(axis=1)
        # repair: for each over-capacity bin, swap a pair of nodes whose
        # degree difference exactly cancels the overflow
        cap = 2 * P
        for _ in range(3000):
            bo = int(np.argmax(sums))
            need = int(sums[bo]) - cap
            if need <= 0:
                break
            db = d[bins[bo]]
            fixed = False
            for io in np.argsort(-db)[:8]:
                d1 = int(db[io])
                d2 = d1 - need
                if d2 < 0:
                    continue
                cand = np.where(sums <= cap - need)[0]
                for bu in cand:
                    hit = np.where(d[bins[bu]] == d2)[0]
                    if len(hit):
                        iu = int(hit[0])
                        bins[bo, io], bins[bu, iu] = bins[bu, iu], bins[bo, io]
                        sums[bo] -= need
                        sums[bu] += need
                        fixed = True
                        break
                if fixed:
                    break
            if not fixed:
                break
        loc = np.empty(NPC, dtype=np.int64)
        loc[bins.reshape(-1)] = np.arange(NB * P)
        loc[heavy] = NB * P + np.arange(len(heavy))
        npos[lo:lo + NPC] = loc

    # table row (256-wide) for node n in the chunked-AG table
    def grow(n):
        o = n // NPC
        r = npos[n]
        k = r // CH_ROWS
        rr = r % CH_ROWS
        return ((k * N_CORES + o) * P + rr % P) * CHM + rr // P

    # ---- layer-1 edge slots: dst-sharded, grouped by dst window ----
    owner = dst // NPC
    win = npos[dst] // P
    cnt = np.zeros((N_CORES, NT), dtype=np.int64)
    np.add.at(cnt, (owner, win), 1)
    tw = np.maximum((cnt.max(axis=0) + P - 1) // P, 1).astype(np.int64)
    ET1 = int(tw.sum())
    pad1 = (-ET1) % GB
    tw[NT - 1] += pad1
    ET1 += pad1
    col_base = np.zeros(NT, dtype=np.int64)
    col_base[1:] = np.cumsum(tw)[:-1]

    gsrc1 = np.zeros((N_CORES, P, ET1), dtype=np.int32)
    gdst1 = np.full((N_CORES, P, ET1), -1.0, dtype=np.float32)

    order = np.lexsort((dst, win, owner))
    so, sw = owner[order], win[order]
    ssrc, sdst = src[order], dst[order]
    grp = so * NT + sw
    gcount = np.zeros(N_CORES * NT, dtype=np.int64)
    np.add.at(gcount, grp, 1)
    starts = np.zeros(N_CORES * NT, dtype=np.int64)
    starts[1:] = np.cumsum(gcount)[:-1]
    pos = np.arange(len(order)) - starts[grp]
    col = col_base[sw] + pos // P
    row = pos % P
    gsrc1[so, row, col] = grow(ssrc).astype(np.int32)
    gdst1[so, row, col] = (npos[sdst] - sw * P).astype(np.float32)

    # ---- layer-2 slots: src-sharded (out-edges), grouped by graph window ----
    gofdst = batch[dst]
    owner2 = src // NPC
    gw = gofdst // P
    cnt2 = np.zeros((N_CORES, NGW), dtype=np.int64)
    np.add.at(cnt2, (owner2, gw), 1)
    tw2 = np.maximum((cnt2.max(axis=0) + P - 1) // P, 1).astype(np.int64)
    ET2 = int(tw2.sum())
    pad2 = (-ET2) % GB
    tw2[NGW - 1] += pad2
    ET2 += pad2
    col_base2 = np.zeros(NGW, dtype=np.int64)
    col_base2[1:] = np.cumsum(tw2)[:-1]

    gsrc2 = np.zeros((N_CORES, P, ET2), dtype=np.int32)
    gdst2 = np.full((N_CORES, P, ET2), -1.0, dtype=np.float32)
    gcf2 = np.zeros((N_CORES, P, ET2), dtype=np.float32)

    # local u-table row index of node j (owner-local): p*NT + w
    def urow(n):
        r = npos[n]
        return (r % P) * NT + r // P

    order2 = np.lexsort((gofdst, gw, owner2))
    so2, sw2 = owner2[order2], gw[order2]
    ssrc2, sdst2, sg2 = src[order2], dst[order2], gofdst[order2]
    grp2 = so2 * NGW + sw2
    gcount2 = np.zeros(N_CORES * NGW, dtype=np.int64)
    np.add.at(gcount2, grp2, 1)
    starts2 = np.zeros(N_CORES * NGW, dtype=np.int64)
    starts2[1:] = np.cumsum(gcount2)[:-1]
    pos2 = np.arange(len(order2)) - starts2[grp2]
    col2 = col_base2[sw2] + pos2 // P
    row2 = pos2 % P
    gsrc2[so2, row2, col2] = urow(ssrc2).astype(np.int32)
    gdst2[so2, row2, col2] = (sg2 - sw2 * P).astype(np.float32)
    gcf2[so2, row2, col2] = dinv[ssrc2] * dinv[sdst2]

    # ---- per-core tensors ----
    w1t = np.ascontiguousarray(
        W1.reshape(K_TILES, P, HID_CH).transpose(1, 0, 2)
    ).astype(bf16)                                        # [128, 6, 256]
    b1t = np.broadcast_to(b1[None, None, :], (P, 1, HID_CH)).astype(bf16)
    b1t = np.ascontiguousarray(b1t)
    iota = np.ascontiguousarray(np.broadcast_to(
        np.arange(P, dtype=np.float32)[None, None, :], (P, 1, P)
    )).astype(bf16)
    ident = np.eye(P, dtype=np.float32).astype(bf16)

    in_maps = []
    for c in range(N_CORES):
        lo = c * NPC
        pc = npos[lo:lo + NPC]
        xs = np.zeros((NPC_PAD, IN_CH), dtype=np.float32)
        xs[pc] = x[lo:lo + NPC]
        # xt[a, m, k, j] = xs[m*128 + j, k*128 + a]
        xt = np.ascontiguousarray(
            xs.reshape(NT, P, K_TILES, P).transpose(3, 0, 2, 1)
        ).astype(bf16)                                    # [128, 196, 6, 128]
        dc = np.zeros(NPC_PAD, dtype=np.float32)
        dc[pc] = dinv[lo:lo + NPC]
        dcol = np.ascontiguousarray(dc.reshape(NT, P).T[:, :, None])
        in_maps.append({
            "xt": xt,
            "w1t": w1t,
            "b1t": b1t,
            "iota": iota,
            "ident": ident,
            "dcol": dcol,
            "gsrc1": np.ascontiguousarray(gsrc1[c]),
            "gdst1": np.ascontiguousarray(gdst1[c])[:, :, None].astype(bf16),
            "gsrc2": np.ascontiguousarray(gsrc2[c]),
            "gdst2": np.ascontiguousarray(gdst2[c])[:, :, None].astype(bf16),
            "gcf2": np.ascontiguousarray(gcf2[c])[:, :, None],
        })

    meta = {"ET1": ET1, "tw": tw.tolist(), "ET2": ET2, "tw2": tw2.tolist()}

    counts = np.bincount(batch, minlength=NUM_GRAPHS).astype(np.int64)
    seg_starts = np.minimum(
        np.searchsorted(batch, np.arange(NUM_GRAPHS)), N_NODES - 1
    )
    aux = {"W2": W2, "b2": b2, "counts": counts, "seg_starts": seg_starts,
           "dinv": dinv, "npos": npos}
    return meta, in_maps, aux


def _build(meta):
    import concourse.bass as bass
    import concourse.bacc as bacc
    import concourse.mybir as mybir
    import concourse.tile as tile

    ET1, tw = meta["ET1"], meta["tw"]
    ET2, tw2 = meta["ET2"], meta["tw2"]
    f32 = mybir.dt.float32
    bf16 = mybir.dt.bfloat16
    i32 = mybir.dt.int32

    nc = bacc.Bacc()
    xt = nc.declare_dram_parameter("xt", [P, NT, K_TILES, P], bf16, isOutput=False)
    w1t_d = nc.declare_dram_parameter("w1t", [P, K_TILES, HID_CH], bf16, isOutput=False)
    b1t_d = nc.declare_dram_parameter("b1t", [P, 1, HID_CH], bf16, isOutput=False)
    iota_d = nc.declare_dram_parameter("iota", [P, 1, P], bf16, isOutput=False)
    ident_d = nc.declare_dram_parameter("ident", [P, P], bf16, isOutput=False)
    dcol_d = nc.declare_dram_parameter("dcol", [P, NT, 1], f32, isOutput=False)
    gsrc1_d = nc.declare_dram_parameter("gsrc1", [P, ET1], i32, isOutput=False)
    gdst1_d = nc.declare_dram_parameter("gdst1", [P, ET1, 1], bf16, isOutput=False)
    gsrc2_d = nc.declare_dram_parameter("gsrc2", [P, ET2], i32, isOutput=False)
    gdst2_d = nc.declare_dram_parameter("gdst2", [P, ET2, 1], bf16, isOutput=False)
    gcf2_d = nc.declare_dram_parameter("gcf2", [P, ET2, 1], f32, isOutput=False)
    u_out = nc.declare_dram_parameter("u_out", [P, NT, HID_CH], bf16, isOutput=True)
    opart = nc.declare_dram_parameter("opart", [P, NGW, HID_CH], f32, isOutput=True)

    f8 = mybir.dt.float8e4
    h1l = nc.dram_tensor("h1l", [NCH, P, CHM, HID_CH], f8)
    h1full = nc.dram_tensor(
        "h1full", [NCH, N_CORES * P, CHM, HID_CH], f8, addr_space="Shared"
    )

    win_of_col1 = []
    for w in range(NT):
        win_of_col1 += [w] * tw[w]
    win_of_col2 = []
    for w in range(NGW):
        win_of_col2 += [w] * tw2[w]

    with tile.TileContext(nc) as tc:
        with (
            tc.tile_pool(name="cb", bufs=1) as cbuf,
            tc.tile_pool(name="sb", bufs=3) as sbuf,
            tc.tile_pool(name="xs", bufs=2) as xbuf,
            tc.tile_pool(name="st", bufs=2) as stg,
            tc.tile_pool(name="ps", bufs=2, space="PSUM") as psum,
            tc.tile_pool(name="ps1", bufs=2, space="PSUM") as psum1,
        ):
            # ---- constants ----
            w1_t = cbuf.tile([P, K_TILES, HID_CH], bf16)
            nc.sync.dma_start(out=w1_t[:], in_=w1t_d[:])
            b1_t = cbuf.tile([P, 1, HID_CH], bf16)
            nc.sync.dma_start(out=b1_t[:], in_=b1t_d[:])
            iota_t = cbuf.tile([P, 1, P], bf16)
            nc.sync.dma_start(out=iota_t[:], in_=iota_d[:])
            ident_t = cbuf.tile([P, P], bf16)
            nc.sync.dma_start(out=ident_t[:], in_=ident_d[:])
            dcol_t = cbuf.tile([P, NT, 1], f32)
            nc.sync.dma_start(out=dcol_t[:], in_=dcol_d[:])
            gsrc1_t = cbuf.tile([P, ET1], i32)
            nc.sync.dma_start(out=gsrc1_t[:], in_=gsrc1_d[:])
            gdst1_t = cbuf.tile([P, ET1, 1], bf16)
            nc.sync.dma_start(out=gdst1_t[:], in_=gdst1_d[:])
            gsrc2_t = cbuf.tile([P, ET2], i32)
            nc.sync.dma_start(out=gsrc2_t[:], in_=gsrc2_d[:])
            gdst2_t = cbuf.tile([P, ET2, 1], bf16)
            nc.sync.dma_start(out=gdst2_t[:], in_=gdst2_d[:])
            gcf2_t = cbuf.tile([P, ET2, 1], f32)
            nc.sync.dma_start(out=gcf2_t[:], in_=gcf2_d[:])
            hloc = cbuf.tile([P, NT, HID_CH], bf16)     # h~ local (SBUF)

            # ---- stage 1: h~ = dinv * (x @ W1) + chunked AllGather ----
            n_mb = (NT + MB - 1) // MB
            next_chunk = 0
            acc1 = None
            for blk in range(n_mb):
                m0 = blk * MB
                mcnt = min(MB, NT - m0)
                xts = xbuf.tile([P, MB, K_TILES, P], bf16, tag="xts")
                nc.sync.dma_start(
                    out=xts[:, 0:mcnt, :, :], in_=xt[:, m0:m0 + mcnt, :, :],
                )
                for mm in range(mcnt):
                    m = m0 + mm
                    sl = m % WB
                    if sl == 0:
                        acc1 = psum1.tile([P, WB, HID_CH], f32, tag="acc1")
                    for k in range(K_TILES):
                        nc.tensor.matmul(
                            acc1[:, sl, :],
                            lhsT=xts[:, mm, k, :],
                            rhs=w1_t[:, k, :],
                            start=(k == 0),
                            stop=(k == K_TILES - 1),
                        )
                    if sl == WB - 1 or m == NT - 1:
                        w0 = m - sl
                        nc.vector.tensor_tensor(
                            out=hloc[:, w0:m + 1, :],
                            in0=acc1[:, 0:sl + 1, :],
                            in1=dcol_t[:, w0:m + 1, :].to_broadcast(
                                [P, sl + 1, HID_CH]
                            ),
                            op=mybir.AluOpType.mult,
                        )
                m_done = m0 + mcnt
                while next_chunk < NCH and m_done >= (next_chunk + 1) * CHM:
                    k = next_chunk
                    # SWDGE cast bf16 -> fp8 during the write
                    nc.gpsimd.dma_start(
                        out=h1l[k], in_=hloc[:, k * CHM:(k + 1) * CHM, :],
                    )
                    nc.gpsimd.collective_compute(
                        "AllGather", mybir.AluOpType.bypass,
                        ins=[h1l[k]], outs=[h1full[k]],
                        replica_groups=[list(range(N_CORES))],
                    )
                    next_chunk += 1

            # ---- layer 1: aggregate -> u table ----
            table1 = h1full[:].rearrange("k r m c -> (k r m) c")
            ustage = None
            accw = None
            for b in range(ET1 // GB):
                c0 = b * GB
                msg = sbuf.tile([P, GB, HID_CH], bf16, tag="msg")
                for t in range(GB):
                    nc.gpsimd.indirect_dma_start(
                        out=msg[:, t, :], out_offset=None,
                        in_=table1,
                        in_offset=bass.IndirectOffsetOnAxis(
                            ap=gsrc1_t[:, c0 + t:c0 + t + 1], axis=0
                        ),
                    )
                pt = sbuf.tile([P, GB, P], bf16, tag="pt")
                nc.vector.tensor_tensor(
                    out=pt[:],
                    in0=gdst1_t[:, c0:c0 + GB, :].to_broadcast([P, GB, P]),
                    in1=iota_t[:].to_broadcast([P, GB, P]),
                    op=mybir.AluOpType.is_equal,
                )
                for t in range(GB):
                    c = c0 + t
                    w = win_of_col1[c]
                    first = (c == 0) or (win_of_col1[c - 1] != w)
                    last = (c == ET1 - 1) or (win_of_col1[c + 1] != w)
                    sl = w % WB
                    if first:
                        if sl == 0:
                            accw = psum.tile([P, WB, HID_CH], f32, tag="accw")
                        nc.tensor.matmul(
                            accw[:, sl, :], lhsT=ident_t[:], rhs=hloc[:, w, :],
                            start=True, stop=False,
                        )
                    nc.tensor.matmul(
                        accw[:, sl, :], lhsT=pt[:, t, :], rhs=msg[:, t, :],
                        start=False, stop=last,
                    )
                    if last and (sl == WB - 1 or w == NT - 1):
                        # u = relu(dinv*acc + b1) for WB windows
                        w0 = w - sl
                        us = w % MB
                        tm = sbuf.tile([P, WB, HID_CH], bf16, tag="tm")
                        nc.vector.tensor_tensor(
                            out=tm[:, 0:sl + 1, :],
                            in0=accw[:, 0:sl + 1, :],
                            in1=dcol_t[:, w0:w + 1, :].to_broadcast(
                                [P, sl + 1, HID_CH]
                            ),
                            op=mybir.AluOpType.mult,
                        )
                        nc.vector.tensor_tensor(
                            out=tm[:, 0:sl + 1, :],
                            in0=tm[:, 0:sl + 1, :],
                            in1=b1_t[:].to_broadcast([P, sl + 1, HID_CH]),
                            op=mybir.AluOpType.add,
                        )
                        if ustage is None:
                            ustage = stg.tile([P, MB, HID_CH], bf16, tag="ustage")
                        nc.scalar.activation(
                            out=ustage[:, us - sl:us + 1, :],
                            in_=tm[:, 0:sl + 1, :],
                            func=mybir.ActivationFunctionType.Relu,
                        )
                        if us == MB - 1 or w == NT - 1:
                            nc.sync.dma_start(
                                out=u_out[:, w - us:w + 1, :],
                                in_=ustage[:, 0:us + 1, :],
                            )
                            ustage = None

            # ---- layer 2 + pooling partials ----
            table2 = u_out[:].rearrange("p w c -> (p w) c")
            ostage = None
            accg = None
            for b in range(ET2 // GB):
                c0 = b * GB
                msg2 = sbuf.tile([P, GB, HID_CH], bf16, tag="msg")
                for t in range(GB):
                    nc.gpsimd.indirect_dma_start(
                        out=msg2[:, t, :], out_offset=None,
                        in_=table2,
                        in_offset=bass.IndirectOffsetOnAxis(
                            ap=gsrc2_t[:, c0 + t:c0 + t + 1], axis=0
                        ),
                    )
                pt2 = sbuf.tile([P, GB, P], bf16, tag="pt")
                nc.vector.tensor_tensor(
                    out=pt2[:],
                    in0=gdst2_t[:, c0:c0 + GB, :].to_broadcast([P, GB, P]),
                    in1=iota_t[:].to_broadcast([P, GB, P]),
                    op=mybir.AluOpType.is_equal,
                )
                nc.vector.tensor_tensor(
                    out=pt2[:],
                    in0=pt2[:],
                    in1=gcf2_t[:, c0:c0 + GB, :].to_broadcast([P, GB, P]),
                    op=mybir.AluOpType.mult,
                )
                for t in range(GB):
                    c = c0 + t
                    w = win_of_col2[c]
                    first = (c == 0) or (win_of_col2[c - 1] != w)
                    last = (c == ET2 - 1) or (win_of_col2[c + 1] != w)
                    sl = w % WB
                    if first and sl == 0:
                        accg = psum.tile([P, WB, HID_CH], f32, tag="accw")
                    nc.tensor.matmul(
                        accg[:, sl, :], lhsT=pt2[:, t, :], rhs=msg2[:, t, :],
                        start=first, stop=last,
                    )
                    if last and (sl == WB - 1 or w == NGW - 1):
                        us = w % MB
                        if ostage is None:
                            ostage = stg.tile([P, MB, HID_CH], f32, tag="ostage")
                        nc.vector.tensor_copy(
                            out=ostage[:, us - sl:us + 1, :],
                            in_=accg[:, 0:sl + 1, :],
                        )
                        if us == MB - 1 or w == NGW - 1:
                            nc.sync.dma_start(
                                out=opart[:, w - us:w + 1, :],
                                in_=ostage[:, 0:us + 1, :],
                            )
                            ostage = None

    nc.finalize()
    return nc


def _host_epilogue(results, aux):
    counts = aux["counts"]
    dinv = aux["dinv"]
    pooled_pre = np.zeros((NG_PAD, HID_CH), dtype=np.float32)
    for c in range(N_CORES):
        op = np.asarray(results[c]["opart"], dtype=np.float32)
        pooled_pre += op.reshape(P, NGW, HID_CH).transpose(1, 0, 2).reshape(
            NG_PAD, HID_CH
        )
    pooled_pre = pooled_pre[:NUM_GRAPHS]
    npos = aux["npos"]
    rs = []
    for c in range(N_CORES):
        u = np.asarray(results[c]["u_out"]).astype(np.float32)
        rows = u.reshape(P, NT, HID_CH).transpose(1, 0, 2).reshape(
            NPC_PAD, HID_CH
        )
        rs.append(rows[npos[c * NPC:(c + 1) * NPC]])
    r = np.concatenate(rs, axis=0)
    v = r * (dinv * dinv)[:, None]
    sums_self = np.add.reduceat(v, aux["seg_starts"], axis=0)
    sums_self[counts == 0] = 0.0
    pooled_pre = pooled_pre + sums_self
    pooled = pooled_pre / np.maximum(counts, 1)[:, None]
    out = pooled.astype(np.float32) @ aux["W2"] + aux["b2"]
    out[counts == 0] = 0.0
    return out.astype(np.float32)


def kernel(**inputs):
    from concourse.bass_utils import run_bass_kernel_spmd

    meta, in_maps, aux = _prep(
        inputs["x"], inputs["W1"], inputs["b1"], inputs["W2"], inputs["b2"],
        inputs["edge_index"], inputs["batch"],
    )
    nc = _build(meta)
    res = run_bass_kernel_spmd(nc, in_maps, list(range(N_CORES)))
    return _host_epilogue(res.results, aux)


# revision 30
# speedup vs baseline: 1.1704x; 1.1341x over previous
"""Trainium2 Bass kernel for DependencyGNN (2-layer GCN + global mean pool).

Distribution (8 cores, SPMD):
  - Nodes sharded contiguously (25000/core, padded to 25088 = 196*128).
  - Stage 1: h~ = dinv * (x_shard @ W1) in bf16, kept in SBUF and written to
    DRAM in 4 chunks; each chunk is AllGathered (bf16) while stage 1 continues.
  - Layer 1 (dst-sharded edges): batched indirect gathers of h~[src] from the
    AG table; one-hot (is_equal) selection matmuls accumulate per 128-dst
    window into PSUM; self-loop added via identity matmul; epilogue
    relu(dinv*acc + b1) -> u table (bf16, local DRAM + kernel output).
  - Layer 2 + mean-pool fused: out-edge (src-local) gathers of u, coefficient
    dinv[dst]*dinv[src], one-hot matmul onto per-graph windows -> per-core
    partial pooled sums (f32 output). No second collective.
  - Host: self-loop pooling terms from u, sum partials, /counts, @W2 + b2.
"""
import numpy as np

N_NODES = 200000
N_EDGES = 400000
IN_CH, HID_CH, OUT_CH = 768, 256, 256
NUM_GRAPHS = 8000
N_CORES = 8
P = 128
NPC = N_NODES // N_CORES            # 25000
NT = (NPC + P - 1) // P             # 196 m-tiles per core
NPC_PAD = NT * P                    # 25088
K_TILES = IN_CH // P                # 6
NCH = 4                             # AllGather chunks
CHM = NT // NCH                     # 49 m-tiles per chunk
CH_ROWS = CHM * P                   # 6272 rows per chunk
NGW = (NUM_GRAPHS + P - 1) // P     # 63 graph windows (8064 padded)
NG_PAD = NGW * P
GB = 16                             # gather batch (tiles per indirect DMA)
MB = 8                              # stage-1 m-tile DMA batch / write batch
WB = 4                              # epilogue window batch (PSUM group)


def _bf16():
    import concourse.mybir as mybir
    return mybir.dt.np(mybir.dt.bfloat16)


def _prep(x, W1, b1, W2, b2, edge_index, batch):
    bf16 = _bf16()
    x = np.asarray(x, dtype=np.float32)
    W1 = np.asarray(W1, dtype=np.float32)
    b1 = np.asarray(b1, dtype=np.float32)
    W2 = np.asarray(W2, dtype=np.float32)
    b2 = np.asarray(b2, dtype=np.float32)
    src = np.asarray(edge_index[0], dtype=np.int64)
    dst = np.asarray(edge_index[1], dtype=np.int64)
    batch = np.asarray(batch, dtype=np.int64)

    deg_in = np.bincount(dst, minlength=N_NODES)
    deg = deg_in.astype(np.float64) + 1.0
    dinv = (1.0 / np.sqrt(deg)).astype(np.float32)

    # ---- per-core node permutation: balance in-edge counts per 128-node
    # window so tw[w] is 2 instead of 3 (fewer padded gather tiles) ----
    NB = NT - 1                       # 195 full bins; bin NT-1 takes the tail
    npos = np.zeros(N_NODES, dtype=np.int64)   # node -> permuted local row
    for c in range(N_CORES):
        lo = c * NPC
        d = deg_in[lo:lo + NPC].astype(np.int64)
        order_desc = np.argsort(-d, kind="stable")
        heavy = order_desc[:NPC - NB * P]      # 40 heaviest -> last window
        rest = order_desc[NPC - NB * P:]       # 24960 = 195*128
        # serpentine deal of degree-sorted rest into NB bins
        rounds = rest.reshape(P, NB)
        cols = np.arange(NB)
        binidx = np.empty((P, NB), dtype=np.int64)
        for r in range(P):
            binidx[r] = cols if r % 2 == 0 else cols[::-1]
        bins = np.full((NB, P), -1, dtype=np.int64)
        for r in range(P):
            bins[binidx[r], r] = rounds[r]
        sums = d[bins].sum(axis=1)
        # repair: for each over-capacity bin, swap a pair of nodes whose
        # degree difference exactly cancels the overflow
        cap = 2 * P
        for _ in range(3000):
            bo = int(np.argmax(sums))
            need = int(sums[bo]) - cap
            if need <= 0:
                break
            db = d[bins[bo]]
            fixed = False
            for io in np.argsort(-db)[:8]:
                d1 = int(db[io])
                d2 = d1 - need
                if d2 < 0:
                    continue
                cand = np.where(sums <= cap - need)[0]
                for bu in cand:
                    hit = np.where(d[bins[bu]] == d2)[0]
                    if len(hit):
                        iu = int(hit[0])
                        bins[bo, io], bins[bu, iu] = bins[bu, iu], bins[bo, io]
                        sums[bo] -= need
                        sums[bu] += need
                        fixed = True
                        break
                if fixed:
                    break
            if not fixed:
                break
        loc = np.empty(NPC, dtype=np.int64)
        loc[bins.reshape(-1)] = np.arange(NB * P)
        loc[heavy] = NB * P + np.arange(len(heavy))
        npos[lo:lo + NPC] = loc

    # table row (256-wide) for node n in the chunked-AG table
    def grow(n):
        o = n // NPC
        r = npos[n]
        k = r // CH_ROWS
        rr = r % CH_ROWS
        return ((k * N_CORES + o) * P + rr % P) * CHM + rr // P

    # ---- layer-1 edge slots: dst-sharded, grouped by dst window ----
    owner = dst // NPC
    win = npos[dst] // P
    cnt = np.zeros((N_CORES, NT), dtype=np.int64)
    np.add.at(cnt, (owner, win), 1)
    tw = np.maximum((cnt.max(axis=0) + P - 1) // P, 1).astype(np.int64)
    ET1 = int(tw.sum())
    pad1 = (-ET1) % GB
    tw[NT - 1] += pad1
    ET1 += pad1
    col_base = np.zeros(NT, dtype=np.int64)
    col_base[1:] = np.cumsum(tw)[:-1]

    gsrc1 = np.zeros((N_CORES, P, ET1), dtype=np.int32)
    gdst1 = np.full((N_CORES, P, ET1), -1.0, dtype=np.float32)

    order = np.lexsort((dst, win, owner))
    so, sw = owner[order], win[order]
    ssrc, sdst = src[order], dst[order]
    grp = so * NT + sw
    gcount = np.zeros(N_CORES * NT, dtype=np.int64)
    np.add.at(gcount, grp, 1)
    starts = np.zeros(N_CORES * NT, dtype=np.int64)
    starts[1:] = np.cumsum(gcount)[:-1]
    pos = np.arange(len(order)) - starts[grp]
    col = col_base[sw] + pos // P
    row = pos % P
    gsrc1[so, row, col] = grow(ssrc).astype(np.int32)
    gdst1[so, row, col] = (npos[sdst] - sw * P).astype(np.float32)

    # ---- layer-2 slots: src-sharded (out-edges), grouped by graph window ----
    gofdst = batch[dst]
    owner2 = src // NPC
    gw = gofdst // P
    cnt2 = np.zeros((N_CORES, NGW), dtype=np.int64)
    np.add.at(cnt2, (owner2, gw), 1)
    tw2 = np.maximum((cnt2.max(axis=0) + P - 1) // P, 1).astype(np.int64)
    ET2 = int(tw2.sum())
    pad2 = (-ET2) % GB
    tw2[NGW - 1] += pad2
    ET2 += pad2
    col_base2 = np.zeros(NGW, dtype=np.int64)
    col_base2[1:] = np.cumsum(tw2)[:-1]

    gsrc2 = np.zeros((N_CORES, P, ET2), dtype=np.int32)
    gdst2 = np.full((N_CORES, P, ET2), -1.0, dtype=np.float32)
    gcf2 = np.zeros((N_CORES, P, ET2), dtype=np.float32)

    # local u-table row index of node j (owner-local): p*NT + w
    def urow(n):
        r = npos[n]
        return (r % P) * NT + r // P

    order2 = np.lexsort((gofdst, gw, owner2))
    so2, sw2 = owner2[order2], gw[order2]
    ssrc2, sdst2, sg2 = src[order2], dst[order2], gofdst[order2]
    grp2 = so2 * NGW + sw2
    gcount2 = np.zeros(N_CORES * NGW, dtype=np.int64)
    np.add.at(gcount2, grp2, 1)
    starts2 = np.zeros(N_CORES * NGW, dtype=np.int64)
    starts2[1:] = np.cumsum(gcount2)[:-1]
    pos2 = np.arange(len(order2)) - starts2[grp2]
    col2 = col_base2[sw2] + pos2 // P
    row2 = pos2 % P
    gsrc2[so2, row2, col2] = urow(ssrc2).astype(np.int32)
    gdst2[so2, row2, col2] = (sg2 - sw2 * P).astype(np.float32)
    gcf2[so2, row2, col2] = dinv[ssrc2] * dinv[sdst2]

    # ---- per-core tensors ----
    w1t = np.ascontiguousarray(
        W1.reshape(K_TILES, P, HID_CH).transpose(1, 0, 2)
    ).astype(bf16)                                        # [128, 6, 256]
    b1t = np.broadcast_to(b1[None, None, :], (P, 1, HID_CH)).astype(bf16)
    b1t = np.ascontiguousarray(b1t)
    iota = np.ascontiguousarray(np.broadcast_to(
        np.arange(P, dtype=np.float32)[None, None, :], (P, 1, P)
    )).astype(bf16)
    ident = np.eye(P, dtype=np.float32).astype(bf16)

    in_maps = []
    for c in range(N_CORES):
        lo = c * NPC
        pc = npos[lo:lo + NPC]
        xs = np.zeros((NPC_PAD, IN_CH), dtype=np.float32)
        xs[pc] = x[lo:lo + NPC]
        # xt[a, m, k, j] = xs[m*128 + j, k*128 + a]
        xt = np.ascontiguousarray(
            xs.reshape(NT, P, K_TILES, P).transpose(3, 0, 2, 1)
        ).astype(bf16)                                    # [128, 196, 6, 128]
        dc = np.zeros(NPC_PAD, dtype=np.float32)
        dc[pc] = dinv[lo:lo + NPC]
        dcol = np.ascontiguousarray(dc.reshape(NT, P).T[:, :, None])
        in_maps.append({
            "xt": xt,
            "w1t": w1t,
            "b1t": b1t,
            "iota": iota,
            "ident": ident,
            "dcol": dcol,
            "gsrc1": np.ascontiguousarray(gsrc1[c]),
            "gdst1": np.ascontiguousarray(gdst1[c])[:, :, None].astype(bf16),
            "gsrc2": np.ascontiguousarray(gsrc2[c]),
            "gdst2": np.ascontiguousarray(gdst2[c])[:, :, None].astype(bf16),
            "gcf2": np.ascontiguousarray(gcf2[c])[:, :, None],
        })

    meta = {"ET1": ET1, "tw": tw.tolist(), "ET2": ET2, "tw2": tw2.tolist()}

    counts = np.bincount(batch, minlength=NUM_GRAPHS).astype(np.int64)
    seg_starts = np.minimum(
        np.searchsorted(batch, np.arange(NUM_GRAPHS)), N_NODES - 1
    )
    aux = {"W2": W2, "b2": b2, "counts": counts, "seg_starts": seg_starts,
           "dinv": dinv, "npos": npos}
    return meta, in_maps, aux


def _build(meta):
    import concourse.bass as bass
    import concourse.bacc as bacc
    import concourse.mybir as mybir
    import concourse.tile as tile

    ET1, tw = meta["ET1"], meta["tw"]
    ET2, tw2 = meta["ET2"], meta["tw2"]
    f32 = mybir.dt.float32
    bf16 = mybir.dt.bfloat16
    i32 = mybir.dt.int32

    nc = bacc.Bacc()
    xt = nc.declare_dram_parameter("xt", [P, NT, K_TILES, P], bf16, isOutput=False)
    w1t_d = nc.declare_dram_parameter("w1t", [P, K_TILES, HID_CH], bf16, isOutput=False)
    b1t_d = nc.declare_dram_parameter("b1t", [P, 1, HID_CH], bf16, isOutput=False)
    iota_d = nc.declare_dram_parameter("iota", [P, 1, P], bf16, isOutput=False)
    ident_d = nc.declare_dram_parameter("ident", [P, P], bf16, isOutput=False)
    dcol_d = nc.declare_dram_parameter("dcol", [P, NT, 1], f32, isOutput=False)
    gsrc1_d = nc.declare_dram_parameter("gsrc1", [P, ET1], i32, isOutput=False)
    gdst1_d = nc.declare_dram_parameter("gdst1", [P, ET1, 1], bf16, isOutput=False)
    gsrc2_d = nc.declare_dram_parameter("gsrc2", [P, ET2], i32, isOutput=False)
    gdst2_d = nc.declare_dram_parameter("gdst2", [P, ET2, 1], bf16, isOutput=False)
    gcf2_d = nc.declare_dram_parameter("gcf2", [P, ET2, 1], f32, isOutput=False)
    u_out = nc.declare_dram_parameter("u_out", [P, NT, HID_CH], bf16, isOutput=True)
    opart = nc.declare_dram_parameter("opart", [P, NGW, HID_CH], f32, isOutput=True)

    f8 = mybir.dt.float8e4
    h1l = nc.dram_tensor("h1l", [NCH, P, CHM, HID_CH], f8)
    h1full = nc.dram_tensor(
        "h1full", [NCH, N_CORES * P, CHM, HID_CH], f8, addr_space="Shared"
    )

    win_of_col1 = []
    for w in range(NT):
        win_of_col1 += [w] * tw[w]
    win_of_col2 = []
    for w in range(NGW):
        win_of_col2 += [w] * tw2[w]

    with tile.TileContext(nc) as tc:
        with (
            tc.tile_pool(name="cb", bufs=1) as cbuf,
            tc.tile_pool(name="sb", bufs=3) as sbuf,
            tc.tile_pool(name="mg", bufs=4) as mbuf,
            tc.tile_pool(name="xs", bufs=2) as xbuf,
            tc.tile_pool(name="st", bufs=2) as stg,
            tc.tile_pool(name="ps", bufs=2, space="PSUM") as psum,
            tc.tile_pool(name="ps1", bufs=2, space="PSUM") as psum1,
        ):
            # ---- constants ----
            w1_t = cbuf.tile([P, K_TILES, HID_CH], bf16)
            nc.sync.dma_start(out=w1_t[:], in_=w1t_d[:])
            b1_t = cbuf.tile([P, 1, HID_CH], bf16)
            nc.sync.dma_start(out=b1_t[:], in_=b1t_d[:])
            iota_t = cbuf.tile([P, 1, P], bf16)
            nc.sync.dma_start(out=iota_t[:], in_=iota_d[:])
            ident_t = cbuf.tile([P, P], bf16)
            nc.sync.dma_start(out=ident_t[:], in_=ident_d[:])
            dcol_t = cbuf.tile([P, NT, 1], f32)
            nc.sync.dma_start(out=dcol_t[:], in_=dcol_d[:])
            gsrc1_t = cbuf.tile([P, ET1], i32)
            nc.sync.dma_start(out=gsrc1_t[:], in_=gsrc1_d[:])
            gdst1_t = cbuf.tile([P, ET1, 1], bf16)
            nc.sync.dma_start(out=gdst1_t[:], in_=gdst1_d[:])
            gsrc2_t = cbuf.tile([P, ET2], i32)
            nc.sync.dma_start(out=gsrc2_t[:], in_=gsrc2_d[:])
            gdst2_t = cbuf.tile([P, ET2, 1], bf16)
            nc.sync.dma_start(out=gdst2_t[:], in_=gdst2_d[:])
            gcf2_t = cbuf.tile([P, ET2, 1], f32)
            nc.sync.dma_start(out=gcf2_t[:], in_=gcf2_d[:])
            hloc = cbuf.tile([P, NT, HID_CH], bf16)     # h~ local (SBUF)

            # ---- stage 1: h~ = dinv * (x @ W1) + chunked AllGather ----
            n_mb = (NT + MB - 1) // MB
            next_chunk = 0
            acc1 = None
            for blk in range(n_mb):
                m0 = blk * MB
                mcnt = min(MB, NT - m0)
                xts = xbuf.tile([P, MB, K_TILES, P], bf16, tag="xts")
                nc.sync.dma_start(
                    out=xts[:, 0:mcnt, :, :], in_=xt[:, m0:m0 + mcnt, :, :],
                )
                for mm in range(mcnt):
                    m = m0 + mm
                    sl = m % WB
                    if sl == 0:
                        acc1 = psum1.tile([P, WB, HID_CH], f32, tag="acc1")
                    for k in range(K_TILES):
                        nc.tensor.matmul(
                            acc1[:, sl, :],
                            lhsT=xts[:, mm, k, :],
                            rhs=w1_t[:, k, :],
                            start=(k == 0),
                            stop=(k == K_TILES - 1),
                        )
                    if sl == WB - 1 or m == NT - 1:
                        w0 = m - sl
                        nc.vector.tensor_tensor(
                            out=hloc[:, w0:m + 1, :],
                            in0=acc1[:, 0:sl + 1, :],
                            in1=dcol_t[:, w0:m + 1, :].to_broadcast(
                                [P, sl + 1, HID_CH]
                            ),
                            op=mybir.AluOpType.mult,
                        )
                m_done = m0 + mcnt
                while next_chunk < NCH and m_done >= (next_chunk + 1) * CHM:
                    k = next_chunk
                    # SWDGE cast bf16 -> fp8 during the write
                    nc.gpsimd.dma_start(
                        out=h1l[k], in_=hloc[:, k * CHM:(k + 1) * CHM, :],
                    )
                    nc.gpsimd.collective_compute(
                        "AllGather", mybir.AluOpType.bypass,
                        ins=[h1l[k]], outs=[h1full[k]],
                        replica_groups=[list(range(N_CORES))],
                    )
                    next_chunk += 1

            # ---- layer 1: aggregate -> u table ----
            table1 = h1full[:].rearrange("k r m c -> (k r m) c")
            ustage = None
            accw = None
            for b in range(ET1 // GB):
                c0 = b * GB
                msg = mbuf.tile([P, GB, HID_CH], bf16, tag="msg")
                for t in range(GB):
                    nc.gpsimd.indirect_dma_start(
                        out=msg[:, t, :], out_offset=None,
                        in_=table1,
                        in_offset=bass.IndirectOffsetOnAxis(
                            ap=gsrc1_t[:, c0 + t:c0 + t + 1], axis=0
                        ),
                    )
                pt = sbuf.tile([P, GB, P], bf16, tag="pt")
                nc.vector.tensor_tensor(
                    out=pt[:],
                    in0=gdst1_t[:, c0:c0 + GB, :].to_broadcast([P, GB, P]),
                    in1=iota_t[:].to_broadcast([P, GB, P]),
                    op=mybir.AluOpType.is_equal,
                )
                for t in range(GB):
                    c = c0 + t
                    w = win_of_col1[c]
                    first = (c == 0) or (win_of_col1[c - 1] != w)
                    last = (c == ET1 - 1) or (win_of_col1[c + 1] != w)
                    sl = w % WB
                    if first:
                        if sl == 0:
                            accw = psum.tile([P, WB, HID_CH], f32, tag="accw")
                        nc.tensor.matmul(
                            accw[:, sl, :], lhsT=ident_t[:], rhs=hloc[:, w, :],
                            start=True, stop=False,
                        )
                    nc.tensor.matmul(
                        accw[:, sl, :], lhsT=pt[:, t, :], rhs=msg[:, t, :],
                        start=False, stop=last,
                    )
                    if last and (sl == WB - 1 or w == NT - 1):
                        # u = relu(dinv*acc + b1) for WB windows
                        w0 = w - sl
                        us = w % MB
                        tm = sbuf.tile([P, WB, HID_CH], bf16, tag="tm")
                        nc.vector.tensor_tensor(
                            out=tm[:, 0:sl + 1, :],
                            in0=accw[:, 0:sl + 1, :],
                            in1=dcol_t[:, w0:w + 1, :].to_broadcast(
                                [P, sl + 1, HID_CH]
                            ),
                            op=mybir.AluOpType.mult,
                        )
                        nc.vector.tensor_tensor(
                            out=tm[:, 0:sl + 1, :],
                            in0=tm[:, 0:sl + 1, :],
                            in1=b1_t[:].to_broadcast([P, sl + 1, HID_CH]),
                            op=mybir.AluOpType.add,
                        )
                        if ustage is None:
                            ustage = stg.tile([P, MB, HID_CH], bf16, tag="ustage")
                        nc.scalar.activation(
                            out=ustage[:, us - sl:us + 1, :],
                            in_=tm[:, 0:sl + 1, :],
                            func=mybir.ActivationFunctionType.Relu,
                        )
                        if us == MB - 1 or w == NT - 1:
                            nc.sync.dma_start(
                                out=u_out[:, w - us:w + 1, :],
                                in_=ustage[:, 0:us + 1, :],
                            )
                            ustage = None

            # ---- layer 2 + pooling partials ----
            table2 = u_out[:].rearrange("p w c -> (p w) c")
            ostage = None
            accg = None
            for b in range(ET2 // GB):
                c0 = b * GB
                msg2 = mbuf.tile([P, GB, HID_CH], bf16, tag="msg")
                for t in range(GB):
                    nc.gpsimd.indirect_dma_start(
                        out=msg2[:, t, :], out_offset=None,
                        in_=table2,
                        in_offset=bass.IndirectOffsetOnAxis(
                            ap=gsrc2_t[:, c0 + t:c0 + t + 1], axis=0
                        ),
                    )
                pt2 = sbuf.tile([P, GB, P], bf16, tag="pt")
                nc.vector.tensor_tensor(
                    out=pt2[:],
                    in0=gdst2_t[:, c0:c0 + GB, :].to_broadcast([P, GB, P]),
                    in1=iota_t[:].to_broadcast([P, GB, P]),
                    op=mybir.AluOpType.is_equal,
                )
                nc.vector.tensor_tensor(
                    out=pt2[:],
                    in0=pt2[:],
                    in1=gcf2_t[:, c0:c0 + GB, :].to_broadcast([P, GB, P]),
                    op=mybir.AluOpType.mult,
                )
                for t in range(GB):
                    c = c0 + t
                    w = win_of_col2[c]
                    first = (c == 0) or (win_of_col2[c - 1] != w)
                    last = (c == ET2 - 1) or (win_of_col2[c + 1] != w)
                    sl = w % WB
                    if first and sl == 0:
                        accg = psum.tile([P, WB, HID_CH], f32, tag="accw")
                    nc.tensor.matmul(
                        accg[:, sl, :], lhsT=pt2[:, t, :], rhs=msg2[:, t, :],
                        start=first, stop=last,
                    )
                    if last and (sl == WB - 1 or w == NGW - 1):
                        us = w % MB
                        if ostage is None:
                            ostage = stg.tile([P, MB, HID_CH], f32, tag="ostage")
                        nc.vector.tensor_copy(
                            out=ostage[:, us - sl:us + 1, :],
                            in_=accg[:, 0:sl + 1, :],
                        )
                        if us == MB - 1 or w == NGW - 1:
                            nc.sync.dma_start(
                                out=opart[:, w - us:w + 1, :],
                                in_=ostage[:, 0:us + 1, :],
                            )
                            ostage = None

    nc.finalize()
    return nc


def _host_epilogue(results, aux):
    counts = aux["counts"]
    dinv = aux["dinv"]
    pooled_pre = np.zeros((NG_PAD, HID_CH), dtype=np.float32)
    for c in range(N_CORES):
        op = np.asarray(results[c]["opart"], dtype=np.float32)
        pooled_pre += op.reshape(P, NGW, HID_CH).transpose(1, 0, 2).reshape(
            NG_PAD, HID_CH
        )
    pooled_pre = pooled_pre[:NUM_GRAPHS]
    npos = aux["npos"]
    rs = []
    for c in range(N_CORES):
        u = np.asarray(results[c]["u_out"]).astype(np.float32)
        rows = u.reshape(P, NT, HID_CH).transpose(1, 0, 2).reshape(
            NPC_PAD, HID_CH
        )
        rs.append(rows[npos[c * NPC:(c + 1) * NPC]])
    r = np.concatenate(rs, axis=0)
    v = r * (dinv * dinv)[:, None]
    sums_self = np.add.reduceat(v, aux["seg_starts"], axis=0)
    sums_self[counts == 0] = 0.0
    pooled_pre = pooled_pre + sums_self
    pooled = pooled_pre / np.maximum(counts, 1)[:, None]
    out = pooled.astype(np.float32) @ aux["W2"] + aux["b2"]
    out[counts == 0] = 0.0
    return out.astype(np.float32)


def kernel(**inputs):
    from concourse.bass_utils import run_bass_kernel_spmd

    meta, in_maps, aux = _prep(
        inputs["x"], inputs["W1"], inputs["b1"], inputs["W2"], inputs["b2"],
        inputs["edge_index"], inputs["batch"],
    )
    nc = _build(meta)
    res = run_bass_kernel_spmd(nc, in_maps, list(range(N_CORES)))
    return _host_epilogue(res.results, aux)
